# revision 35
# baseline (speedup 1.0000x reference)
"""Dense 3-layer GAT on 8 TRN2 NeuronCores.

Sharding: each core owns 512 query nodes (rows of the attention score
matrix). Per layer, each core computes h = x @ W for its own nodes,
AllGathers h (fp8e4) per head across the 8 cores, then computes its
512-query slab of masked-softmax attention and the attended output.

f_src (query-side attention bias) is always local. f_dst (key side) for
layer 0 is computed locally from a replicated full x^T input (no
collective on the layer-0 critical path); for layers 1/2 it is
AllGathered (tiny [H,512] f32).

Queue discipline: the gpsimd queue carries collectives plus the DMAs
that must wait on them (h stores -> gather(h) -> hg load(h), issued
interleaved per head so the gather pipeline never stalls); the sync
queue carries all input/weight DMAs and the f_src partition-broadcast
DMAs. Scores are stored as exp(lrelu(z) - 4) in fp8e5 (the global shift
cancels in the softmax normalize and keeps the fp8 range safe), so the
attention matmuls run in DoubleRow fp8 mode (2 key blocks per matmul).
Score generation is split ~50/50 between the Scalar (ACT) and Vector
(DVE) engines.

kernel(**inputs) takes the full unsharded inputs and returns the full
[4096, 256] output.
"""

from contextlib import ExitStack

import numpy as np
import ml_dtypes

import concourse.mybir as mybir
import concourse.tile as tile
from concourse import bacc
from concourse.bass_utils import run_bass_kernel_spmd
from concourse.masks import make_identity

P = 128
N_NODES = 4096
S = 512                    # nodes per core
NB = N_NODES // P          # 32 global key blocks
H = 4
LAYERS = [(512, 512), (2048, 512), (2048, 64)]
F32 = mybir.dt.float32
F32R = mybir.dt.float32r
BF16 = mybir.dt.bfloat16
F8E4 = mybir.dt.float8e4
F8E5 = mybir.dt.float8e5
AF = mybir.ActivationFunctionType
ALU = mybir.AluOpType
DR = mybir.MatmulPerfMode.DoubleRow

EXP_SHIFT = -4.0           # exp(lrelu(z) - 4): cancels in normalize

_CACHE = {}


def _build():
    nc = bacc.Bacc("TRN2", target_bir_lowering=False, debug=False, num_devices=8)

    xT0_d = nc.dram_tensor("xT0", [512, S], BF16, kind="ExternalInput")
    xTall_d = nc.dram_tensor("xTall", [512, N_NODES], BF16, kind="ExternalInput")
    adjT_d = nc.dram_tensor("adjT", [N_NODES, S], BF16, kind="ExternalInput")
    W_d = []
    WA_d = []
    for li, (fin, fout) in enumerate(LAYERS):
        W_d.append(nc.dram_tensor(f"W{li}", [fin, H * fout], BF16, kind="ExternalInput"))
        # cols 0..3 = src heads, cols 4..7 = dst heads
        WA_d.append(nc.dram_tensor(f"WA{li}", [fin, 2 * H], BF16, kind="ExternalInput"))
    outT_d = nc.dram_tensor("outT", [H * 64, S], F32, kind="ExternalOutput")

    with tile.TileContext(nc) as tc:
        with ExitStack() as ctx:
            constp = ctx.enter_context(tc.tile_pool(name="const", bufs=1))
            adjp = ctx.enter_context(tc.tile_pool(name="adjp", bufs=1))
            xbp = ctx.enter_context(tc.tile_pool(name="xb", bufs=21))
            xtallp = ctx.enter_context(tc.tile_pool(name="xtall", bufs=3))
            xtp = ctx.enter_context(tc.tile_pool(name="xt", bufs=4))
            wtp = ctx.enter_context(tc.tile_pool(name="wt", bufs=16))
            wap = ctx.enter_context(tc.tile_pool(name="wap", bufs=17))
            hfp = ctx.enter_context(tc.tile_pool(name="hfp", bufs=2))
            hgp = ctx.enter_context(tc.tile_pool(name="hgp", bufs=4))
            scp = ctx.enter_context(tc.tile_pool(name="scp", bufs=3))
            stp = ctx.enter_context(tc.tile_pool(name="stp", bufs=20))
            zp = ctx.enter_context(tc.tile_pool(name="zp", bufs=2))
            fbp = ctx.enter_context(tc.tile_pool(name="fbp", bufs=1))
            fdp = ctx.enter_context(tc.tile_pool(name="fdp", bufs=2))
            evp = ctx.enter_context(tc.tile_pool(name="evp", bufs=2))
            rcp = ctx.enter_context(tc.tile_pool(name="rcp", bufs=1))
            frp = ctx.enter_context(tc.tile_pool(name="frp", bufs=4))
            psO = ctx.enter_context(tc.tile_pool(name="psO", bufs=5, space="PSUM"))
            psH = ctx.enter_context(tc.tile_pool(name="psH", bufs=2, space="PSUM"))
            psR = ctx.enter_context(tc.tile_pool(name="psR", bufs=1, space="PSUM"))
            dr = ctx.enter_context(tc.tile_pool(name="dram", bufs=1, space="DRAM"))

            # dummy collective first in the gpsimd queue: absorbs the ncfw
            # first-call staging cost during kernel-init and layer-0 work
            warm_in = dr.tile([P, 4], F32, tag="warm_in")
            warm_out = dr.tile([8, P, 4], F32, tag="warm_out", addr_space="Shared")
            nc.gpsimd.collective_compute(
                "AllGather", ALU.bypass,
                replica_groups=[list(range(8))],
                ins=[warm_in[:].opt()], outs=[warm_out[:].opt()],
            )
            ident = constp.tile([P, P], F32, tag="ident")
            make_identity(nc, ident[:])
            ones_r = constp.tile([1, P], F32, tag="ones_r")
            nc.any.memset(ones_r[:], 1.0)
            # pair-dim stride must be 0 mod 16 elements for DoubleRow LDWEIGHTS
            ones_dr_t = constp.tile([P, 2, 16], F8E5, tag="ones_dr")
            nc.any.memset(ones_dr_t[:], 1.0)
            ones_dr = ones_dr_t[:, :, 0:1]
            shift_c = constp.tile([P, 1], F32, tag="shift_c")
            nc.any.memset(shift_c[:], EXP_SHIFT)
            ones_rb = constp.tile([1, P], BF16, tag="ones_rb")
            nc.any.memset(ones_rb[:], 1.0)
            ones_rr = constp.tile([1, P], F32R, tag="ones_rr")
            nc.any.memset(ones_rr[:].bitcast(F32), 1.0)

            # layer-0 x^T first (A1 needs it); the big adjacency slab
            # (additive mask, bf16) isn't read until the first score tiles
            xb_cur = []
            for kb in range(4):
                tb = xbp.tile([P, S], BF16, tag="xb")
                nc.sync.dma_start(tb[:], xT0_d[kb * P:(kb + 1) * P, :])
                xb_cur.append(tb)
            adjT_res = adjp.tile([P, NB, S], BF16, tag="adjT")

            def load_adjT():
                for rr in range(8):
                    nc.sync.dma_start(
                        adjT_res[:, 4 * rr:4 * rr + 4, :],
                        adjT_d[4 * rr * P:4 * (rr + 1) * P, :]
                        .rearrange("(nb p) n -> p nb n", p=P),
                    )

            for li, (fin, fout) in enumerate(LAYERS):
                KB = fin // P
                HB = S * fout          # bytes of one head's h (fp8)
                FB = H * S * 4         # bytes of the f_dst block (f32)
                if li == 0:
                    # two head-pair gathers, f_dst is local
                    agh_in = [dr.tile([2, S, fout], F8E4, tag=f"aghi0_{pp}",
                                      name=f"aghi0_{pp}")
                              for pp in range(2)]
                    agh_out = [dr.tile([8, 2, S, fout], F8E4,
                                       tag=f"agho0_{pp}", name=f"agho0_{pp}",
                                       addr_space="Shared")
                               for pp in range(2)]
                elif li == 1:
                    # G1 = [h0, h1, f_dst bytes], G2 = [h2, h3]
                    agh_in = [dr.tile([2 * HB + FB], F8E4, tag="aghi1_0",
                                      name="aghi1_0"),
                              dr.tile([2 * HB], F8E4, tag="aghi1_1",
                                      name="aghi1_1")]
                    agh_out = [dr.tile([8, 2 * HB + FB], F8E4, tag="agho1_0",
                                       name="agho1_0", addr_space="Shared"),
                               dr.tile([8, 2 * HB], F8E4, tag="agho1_1",
                                       name="agho1_1", addr_space="Shared")]
                else:
                    # single fused gather: [h(all heads), f_dst bytes]
                    agh_in = [dr.tile([H * HB + FB], F8E4, tag="aghi2",
                                      name="aghi2")]
                    agh_out = [dr.tile([8, H * HB + FB], F8E4, tag="agho2",
                                       name="agho2", addr_space="Shared")]
                if li > 0:
                    agf_in = dr.tile([H, S], F32, tag=f"agfi{li}",
                                     name=f"agfi{li}")
                    agf_out = dr.tile([8, H, S], F32, tag=f"agfo{li}",
                                      name=f"agfo{li}", addr_space="Shared")

                # ---- phase A1: f projections ----
                # fT_sb[ml, mh, r, j] = f_dst[j, r*512 + mh*128 + ml]
                fT_sb = fdp.tile([P, 4, 8, H], F32, tag="fdst", name=f"fts{li}")
                fsrc_bf = fdp.tile([H, S], BF16, tag="fsrcbf", name=f"fsb{li}")
                if li == 0:
                    was_t, wad_t = [], []
                    for kb in range(4):
                        t = wap.tile([P, H], BF16, tag="wa", name="was0")
                        nc.sync.dma_start(t[:], WA_d[0][kb * P:(kb + 1) * P, 0:H])
                        was_t.append(t)
                        t2 = wap.tile([P, H], BF16, tag="wa", name="wad0")
                        nc.sync.dma_start(t2[:], WA_d[0][kb * P:(kb + 1) * P, H:2 * H])
                        wad_t.append(t2)
                    # src: from own x slab
                    pfs = psH.tile([H, S], F32, tag="ph", name="pfs0")
                    for kb in range(4):
                        nc.tensor.matmul(pfs[:], was_t[kb][:], xb_cur[kb][:],
                                         start=(kb == 0), stop=(kb == 3))
                    nc.vector.tensor_copy(fsrc_bf[:], pfs[:])
                    # dst: from replicated full x^T, streamed in 2
                    # buffers. 8 rank-slab chunks, each a [4,512] psum
                    # accumulation chain; chunks sit at base partitions
                    # 0/32/64 of 3 banks so their psum zero regions (which
                    # are per partition row) never collide while the
                    # per-kb chains interleave.
                    ptf = [psO.tile([P, S], F32, tag="po", name=f"ptf{i}")
                           for i in range(3)]
                    for kb in range(4):
                        for half in range(2):
                            xt_buf = xtallp.tile([P, N_NODES // 2], BF16,
                                                 tag="xta")
                            nc.sync.dma_start(
                                xt_buf[:],
                                xTall_d[kb * P:(kb + 1) * P,
                                        half * 2048:(half + 1) * 2048])
                            for c in range(half * 4, half * 4 + 4):
                                base = (c % 3) * 32
                                nc.tensor.matmul(
                                    ptf[c // 3][base:base + H, :],
                                    wad_t[kb][:],
                                    xt_buf[:, (c % 4) * S:(c % 4 + 1) * S],
                                    start=(kb == 0), stop=(kb == 3),
                                )
                    # per-rank chunk copies (so transposes read from
                    # base partition 0), then PE transposes into ptd0 with
                    # columns ordered (mh, r, j)
                    ptd0 = psH.tile([P, P], F32, tag="ph", name="ptd0")
                    for r in range(8):
                        base = (r % 3) * 32
                        fd0r = fdp.tile([H, S], F32, tag="fgat",
                                        name=f"fd0r{r}")
                        nc.scalar.copy(fd0r[:], ptf[r // 3][base:base + H, :])
                        for mh in range(4):
                            nc.tensor.transpose(
                                ptd0[:, (mh * 8 + r) * H:(mh * 8 + r + 1) * H],
                                fd0r[:, mh * P:(mh + 1) * P],
                                ident[0:H, 0:H],
                            )
                    nc.scalar.copy(
                        fT_sb[:],
                        ptd0[:].rearrange("p (mh rj) -> p mh rj", mh=4)
                        .rearrange("p mh (r j) -> p mh r j", r=8),
                    )
                else:
                    wa_tiles = []
                    for kb in range(KB):
                        t = wap.tile([P, 2 * H], BF16, tag="wa", name=f"wa{li}")
                        nc.sync.dma_start(t[:], WA_d[li][kb * P:(kb + 1) * P, :])
                        wa_tiles.append(t)
                    pf = psH.tile([2 * H, S], F32, tag="ph", name=f"pf{li}")
                    for kb in range(KB):
                        nc.tensor.matmul(pf[:], wa_tiles[kb][:], xb_cur[kb][:],
                                         start=(kb == 0), stop=(kb == KB - 1))
                    fT_loc = fdp.tile([2 * H, S], F32, tag="ftl", name=f"ftl{li}")
                    nc.scalar.copy(fT_loc[:], pf[:])
                    nc.vector.tensor_copy(fsrc_bf[:], fT_loc[0:H, :])
                    nc.gpsimd.dma_start(agf_in[:], fT_loc[H:2 * H, :])
                    nc.gpsimd.collective_compute(
                        "AllGather", ALU.bypass,
                        replica_groups=[list(range(8))],
                        ins=[agf_in[:].opt()], outs=[agf_out[:].opt()],
                    )

                def emit_c2(li=li, agf_out=agf_out if li > 0 else None,
                            fT_sb=fT_sb):
                    # fgat row (r*H + j) col m = f_dst[j, r*512 + m]
                    fgat = fdp.tile([NB, S], F32, tag="fgat")
                    nc.sync.dma_start(
                        fgat[:], agf_out[:].rearrange("r j m -> (r j) m")
                    )
                    ptd = psH.tile([P, P], F32, tag="ph", name=f"ptd{li}")
                    for mh in range(4):
                        nc.tensor.transpose(
                            ptd[:, mh * NB:(mh + 1) * NB],
                            fgat[:, mh * P:(mh + 1) * P],
                            ident[0:NB, 0:NB],
                        )
                    nc.scalar.copy(
                        fT_sb[:],
                        ptd[:].rearrange("p (mh rj) -> p mh rj", mh=4)
                        .rearrange("p mh (r j) -> p mh r j", r=8),
                    )

                # f_src broadcast: rank-1 bf16 PE matmul per head.
                # The row-staging DMAs are issued here (A1) so they don't
                # queue behind the big W/adjT triggers on the sync queue.
                fsb_bcast = fbp.tile([P, H, S], BF16, tag="fsb_b")
                fr_rows = []
                for h in range(H):
                    fr = frp.tile([1, S], BF16, tag="frb",
                                  name=f"fr{li}_{h}")
                    nc.sync.dma_start(fr[:], fsrc_bf[h:h + 1, :])
                    fr_rows.append(fr)

                def emit_fsb(li=li, fsb_bcast=fsb_bcast, fr_rows=fr_rows):
                    for h in range(H):
                        pb = psH.tile([P, S], F32, tag="ph",
                                      name=f"pb{li}_{h}")
                        nc.tensor.matmul(pb[:], ones_rb[:], fr_rows[h][:],
                                         start=True, stop=True)
                        nc.vector.tensor_copy(fsb_bcast[:, h, :], pb[:])

                nob = 1 if fout == 64 else 4
                rows = 64 if fout == 64 else P
                st_tiles = {}

                def emit_scores(h, li=li, st_tiles=st_tiles,
                                fsb_bcast=fsb_bcast, fT_sb=fT_sb,
                                adjT_res=adjT_res):
                    for r in range(8):
                        lr4 = scp.tile([P, 4, S], BF16, tag="lr",
                                       name=f"lr{li}")
                        for i in range(4):
                            mb = 4 * r + i
                            bias_f = fT_sb[:, i, r, h:h + 1]
                            if mb % 2 == 0:
                                z = zp.tile([P, S], BF16, tag="z",
                                            name=f"z{li}")
                                nc.vector.tensor_scalar(
                                    z[:], fsb_bcast[:, h, :], bias_f, None,
                                    ALU.add,
                                )
                                nc.vector.scalar_tensor_tensor(
                                    lr4[:, i, :], z[:], 0.2, z[:],
                                    ALU.mult, ALU.max,
                                )
                            else:
                                nc.scalar.activation(
                                    lr4[:, i, :], fsb_bcast[:, h, :],
                                    AF.Prelu, bias=bias_f, scale=1.0,
                                    alpha=0.2,
                                )
                        lrm = scp.tile([P, 4, S], BF16, tag="lm",
                                       name=f"lm{li}")
                        nc.vector.tensor_tensor(
                            lrm[:], lr4[:],
                            adjT_res[:, 4 * r:4 * r + 4, :],
                            ALU.add,
                        )
                        st4 = stp.tile([P, 4, S], F8E5, tag="st",
                                       name=f"st{li}")
                        nc.scalar.activation(st4[:], lrm[:], AF.Exp,
                                             bias=shift_c[:], scale=1.0)
                        st_tiles[(h, r)] = st4

                # ---- phase A2: h = x @ W (bf16 -> fp8e4), 2 gathers ----
                def load_hg(h, r, li=li, fout=fout, agh_out=agh_out, HB=HB):
                    hg = hgp.tile([P, 4, fout], F8E4, tag="hg")
                    if li == 0:
                        src = agh_out[h // 2][r, h % 2].rearrange(
                            "(b p) f -> p b f", p=P)
                    elif li == 1:
                        src = agh_out[h // 2][r, (h % 2) * HB:
                                              (h % 2) * HB + HB].rearrange(
                            "(b p f) -> p b f", p=P, b=4)
                    else:
                        src = agh_out[0][r, h * HB:(h + 1) * HB].rearrange(
                            "(b p f) -> p b f", p=P, b=4)
                    nc.sync.dma_start(hg[:], src)
                    return hg

                def gather(pp, li=li, agh_in=agh_in, agh_out=agh_out):
                    nc.gpsimd.collective_compute(
                        "AllGather", ALU.bypass,
                        replica_groups=[list(range(8))],
                        ins=[agh_in[pp][:].opt()],
                        outs=[agh_out[pp][:].opt()],
                    )

                if li < 2:
                    w_tiles = {}
                    for h in range(H):
                        for kb in range(KB):
                            t = wtp.tile([P, fout], BF16, tag="wt",
                                         name=f"w{li}_{h}_{kb}")
                            nc.sync.dma_start(
                                t[:],
                                W_d[li][kb * P:(kb + 1) * P,
                                        h * fout:(h + 1) * fout],
                            )
                            w_tiles[(h, kb)] = t

                    def a2_head(h, li=li, KB=KB, fout=fout, HB=HB,
                                w_tiles=w_tiles, xb_cur=xb_cur,
                                agh_in=agh_in):
                        h_sb = hfp.tile([P, 4, fout], F8E4, tag="hsb")
                        for b in range(4):
                            ph = psH.tile([P, S], F32, tag="ph",
                                          name=f"ph{li}_{h}_{b}")
                            for kb in range(KB):
                                nc.tensor.matmul(
                                    ph[:, 0:fout],
                                    xb_cur[kb][:, b * P:(b + 1) * P],
                                    w_tiles[(h, kb)][:],
                                    start=(kb == 0), stop=(kb == KB - 1),
                                )
                            nc.vector.tensor_copy(h_sb[:, b, :], ph[:, 0:fout])
                        if li == 0:
                            dst = agh_in[h // 2][h % 2].rearrange(
                                "(b p) f -> p b f", p=P)
                        else:
                            dst = agh_in[h // 2][(h % 2) * HB:
                                                 (h % 2) * HB + HB].rearrange(
                                "(b p f) -> p b f", p=P, b=4)
                        nc.gpsimd.dma_start(dst, h_sb[:])

                    if li == 0:
                        load_adjT()
                    emit_fsb()
                    a2_head(0)
                    if li > 0:
                        emit_c2()
                    a2_head(1)
                    gather(0)
                    emit_scores(0)
                    a2_head(2)
                    a2_head(3)
                    gather(1)
                    emit_scores(1)
                    emit_scores(2)
                else:
                    # merged: one rhs covers all 4 heads (4*64 = 256 cols)
                    w_tiles2 = []
                    for kb in range(KB):
                        t = wtp.tile([P, H * fout], BF16, tag="wt",
                                     name=f"w2_{kb}")
                        nc.sync.dma_start(
                            t[:], W_d[li][kb * P:(kb + 1) * P, :]
                        )
                        w_tiles2.append(t)
                    emit_fsb()
                    for b in range(4):
                        ph = psH.tile([P, H * fout], F32, tag="ph",
                                      name=f"ph{li}_{b}")
                        for kb in range(KB):
                            nc.tensor.matmul(
                                ph[:, 0:H * fout],
                                xb_cur[kb][:, b * P:(b + 1) * P],
                                w_tiles2[kb][:],
                                start=(kb == 0), stop=(kb == KB - 1),
                            )
                        h_sb = hfp.tile([P, H * fout], F8E4, tag="hsb")
                        nc.vector.tensor_copy(h_sb[:], ph[:, 0:H * fout])
                        nc.gpsimd.dma_start(
                            agh_in[0][0:H * HB]
                            .rearrange("(h s f) -> h s f", h=H, s=S)
                            [:, b * P:(b + 1) * P, :]
                            .rearrange("h p f -> p h f"),
                            h_sb[:].rearrange("p (h f) -> p h f", h=H),
                        )
                    gather(0)
                    emit_c2()
                    emit_scores(0)
                    emit_scores(1)

                # ---- phase D: attention (DoubleRow fp8) + normalize/ELU ----
                xt_next = []
                xb_next = []
                for h in range(H):
                    po = [
                        psO.tile([P, S], F32, tag="po", name=f"po{li}_{h}_{ob}")
                        for ob in range(nob)
                    ]
                    prs = psR.tile([1, S], F32, tag="prs", name=f"prs{li}_{h}")
                    for r in range(8):
                        hg = load_hg(h, r)
                        st4 = st_tiles.pop((h, r))
                        for i2 in range(2):
                            pi = 2 * r + i2
                            st_pair = st4[:, 2 * i2:2 * i2 + 2, :]
                            for ob in range(nob):
                                lhsT = (hg[:, 2 * i2:2 * i2 + 2,
                                           ob * P:(ob + 1) * P]
                                        if fout != 64
                                        else hg[:, 2 * i2:2 * i2 + 2, :])
                                nc.tensor.matmul(
                                    po[ob][0:rows, :], lhsT, st_pair,
                                    start=(pi == 0), stop=(pi == 15),
                                    perf_mode=DR,
                                )
                            nc.tensor.matmul(
                                prs[:], ones_dr, st_pair,
                                start=(pi == 0), stop=(pi == 15),
                                perf_mode=DR,
                            )

                    # normalize + ELU; PSUM is read directly by DVE.
                    # max() guards rowsum against fp8-flushed columns.
                    rsum = rcp.tile([1, S], F32R, tag="rsum")
                    nc.vector.tensor_scalar(rsum[:], prs[:], 1e-6, None, ALU.max)
                    pb2 = psH.tile([P, S], F32, tag="ph", name=f"pb2{li}_{h}")
                    nc.tensor.matmul(pb2[:], ones_rr[:], rsum[:],
                                     start=True, stop=True)
                    rb = rcp.tile([P, S], F32, tag="rb")
                    nc.vector.reciprocal_approx_fast(rb[:], pb2[:])
                    for ob in range(nob):
                        t0 = evp.tile([rows, S], F32, tag="t0")
                        nc.vector.tensor_tensor(t0[:], po[ob][0:rows, :],
                                                rb[0:rows, :], ALU.mult)
                        # elu(x) = min(exp(x) - 1, relu(x))
                        em = evp.tile([rows, S], F32, tag="em")
                        nc.scalar.activation(em[:], t0[:], AF.Exp, bias=0.0, scale=1.0)
                        rl = evp.tile([rows, S], F32, tag="rl")
                        nc.vector.tensor_scalar_max(rl[:], t0[:], 0.0)
                        if li < 2:
                            xbn = xbp.tile([rows, S], BF16, tag="xb", name=f"xb{li}")
                            nc.vector.scalar_tensor_tensor(
                                xbn[:], em[:], -1.0, rl[:], ALU.add, ALU.min
                            )
                            xb_next.append(xbn)
                        else:
                            xnt = evp.tile([rows, S], F32, tag="t0", name=f"xn{li}")
                            nc.vector.scalar_tensor_tensor(
                                xnt[:], em[:], -1.0, rl[:], ALU.add, ALU.min
                            )
                            em2 = evp.tile([rows, S], F32, tag="em", name=f"em2{li}")
                            nc.scalar.activation(em2[:], xnt[:], AF.Exp,
                                                 bias=0.0, scale=1.0)
                            rl2 = evp.tile([rows, S], F32, tag="rl", name=f"rl2{li}")
                            nc.vector.tensor_scalar_max(rl2[:], xnt[:], 0.0)
                            x2 = xtp.tile([rows, S], F32, tag="xt", name=f"x2{li}")
                            nc.vector.scalar_tensor_tensor(
                                x2[:], em2[:], -1.0, rl2[:], ALU.add, ALU.min
                            )
                            nc.sync.dma_start(
                                outT_d[h * 64:(h + 1) * 64, :], x2[:])
                            xt_next.append(x2)
                    if (h + 2 < H and fout == 64) or h + 3 == H:
                        emit_scores(h + 2)

                xb_cur = xb_next

            # outputs were DMA'd per head inside the L2 loop

    nc.compile()
    return nc


def build_in_maps(inputs):
    node_feats = np.ascontiguousarray(inputs["node_feats"], dtype=np.float32)
    adj = np.asarray(inputs["adj"], dtype=np.float32)
    Ws = [np.asarray(inputs[f"W{i}"], dtype=np.float32) for i in range(3)]
    As = [np.asarray(inputs[f"a{i}"], dtype=np.float32) for i in range(3)]

    WAs = []
    Wcats = []
    for W, a in zip(Ws, As):
        # cols 0..3 = src heads (a[:,0]), cols 4..7 = dst heads (a[:,1])
        wsrc = np.einsum("hfo,ho->fh", W.astype(np.float64),
                         a[:, 0].astype(np.float64))
        wdst = np.einsum("hfo,ho->fh", W.astype(np.float64),
                         a[:, 1].astype(np.float64))
        wa = np.concatenate([wsrc, wdst], axis=1).astype(ml_dtypes.bfloat16)
        WAs.append(np.ascontiguousarray(wa))
        wcat = np.ascontiguousarray(
            np.transpose(W, (1, 0, 2)).reshape(W.shape[1], -1)
        ).astype(ml_dtypes.bfloat16)
        Wcats.append(wcat)

    xT_all = np.ascontiguousarray(node_feats.T).astype(ml_dtypes.bfloat16)
    in_maps = []
    for c in range(8):
        rows = slice(c * S, (c + 1) * S)
        m = {
            "xT0": np.ascontiguousarray(node_feats[rows].T).astype(ml_dtypes.bfloat16),
            "xTall": xT_all,
            "adjT": np.ascontiguousarray((adj[rows].T - 1.0) * 50.0).astype(ml_dtypes.bfloat16),
        }
        for i in range(3):
            m[f"W{i}"] = Wcats[i]
            m[f"WA{i}"] = WAs[i]
        in_maps.append(m)
    return in_maps


def kernel(**inputs):
    if "nc" not in _CACHE:
        _CACHE["nc"] = _build()
    nc = _CACHE["nc"]
    in_maps = build_in_maps(inputs)
    res = run_bass_kernel_spmd(nc, in_maps, core_ids=list(range(8)))
    out = np.concatenate([r["outT"].T for r in res.results], axis=0)
    return np.ascontiguousarray(out, dtype=np.float32)


if __name__ == "__main__":
    rng = np.random.default_rng(0)
    fake = {
        "node_feats": rng.standard_normal((N_NODES, 512), dtype=np.float32),
        "edge_feats": rng.standard_normal((131072, 16), dtype=np.float32),
        "edge_indices": rng.integers(0, N_NODES, (2, 131072)).astype(np.int32),
        "adj": np.maximum(
            (rng.random((N_NODES, N_NODES)) < 0.01).astype(np.float32),
            np.eye(N_NODES, dtype=np.float32),
        ),
    }
    for i, (fin, fout) in enumerate(LAYERS):
        fake[f"W{i}"] = (rng.standard_normal((H, fin, fout)) * 0.05).astype(np.float32)
        fake[f"a{i}"] = (rng.standard_normal((H, 2, fout)) * 0.05).astype(np.float32)
    o = kernel(**fake)
    print("kernel output", o.shape, o.dtype, np.abs(o).mean())
